# revision 1
# baseline (speedup 1.0000x reference)
"""Block-sparse linear kernel for Trainium2 (8 NeuronCores, SPMD data-parallel).

Computes y = x @ (W * mask) + bias for
    x    [8, 1024, 4096] f32
    W    [4096, 4096]    f32
    mask [4096, 4096]    int32 (32x32-block structured, ~25% block density)
    bias [4096]          f32
    y    [8, 1024, 4096] f32

Strategy
--------
- Data parallel: core c computes rows [1024c, 1024(c+1)) of the flattened
  [8192, 4096] activation (i.e. batch element c).
- The PE array on trn2 is physically 16 independent 32x32 sub-arrays
  addressable via `tile_position`.  The mask's 32x32 block granularity maps
  exactly onto that: each nonzero weight block (i, j) becomes one
  K=32/M=32/N=512 matmul on sub-array (row_grp=i%4, col_grp=j%4); all-zero
  blocks are simply skipped, cutting PE work to the block density.
- Row groups write disjoint PSUM banks (bank = row group) so the 16
  sub-arrays never collide on a PSUM bank; VectorE reduces the 4 partial
  banks and the result is DMA'd out.
- Weights are gathered host-side into per-row-strip BSR-style panels
  (this mirrors the nn.Module, which stores BSR values at init), cast to
  bf16; x is transposed/cast host-side.  The device does all the matmul
  FLOPs in bf16 with fp32 accumulation.
- The device program is compiled against the observed block pattern; it is
  exact for arbitrary masks (any block containing a nonzero mask element is
  gathered with its W*mask values; empty blocks contribute exact zeros).
"""

import numpy as np
import ml_dtypes

B, S, IN_F, OUT_F = 8, 1024, 4096, 4096
BS = 32                      # sparsity block size
GI, GJ = IN_F // BS, OUT_F // BS
N_CORES = 8
M_CORE = (B * S) // N_CORES  # rows of x per core (1024)
MSL = 512                    # m-slice width (one PSUM bank of fp32)
N_MSL = M_CORE // MSL        # 2
JCOLS = 4                    # output block-columns per supertile (4*32 = 128 partitions)
N_J = GJ // JCOLS            # 32 output supertiles
N_T = IN_F // 128            # 32 xT tiles

BF16 = ml_dtypes.bfloat16


def _ensure_ntff_hook():
    """Best-effort: make trace=True work under axon when the image's antenv
    lacks axon_hooks.  Harmless if it fails — tracing is skipped, results
    are still correct."""
    import sys, types
    try:
        import antenv  # noqa
    except ImportError:
        return
    try:
        from antenv.axon_hooks import get_axon_ntff_profile_hook
        if get_axon_ntff_profile_hook() is not None:
            return
        mod = sys.modules["antenv.axon_hooks"]
    except ImportError:
        mod = types.ModuleType("antenv.axon_hooks")
        mod._hook = None
        def set_axon_ntff_profile_hook(h, _m=mod):
            _m._hook = h
        def get_axon_ntff_profile_hook(_m=mod):
            return _m._hook
        mod.set_axon_ntff_profile_hook = set_axon_ntff_profile_hook
        mod.get_axon_ntff_profile_hook = get_axon_ntff_profile_hook
        sys.modules["antenv.axon_hooks"] = mod
        import antenv as _a
        _a.axon_hooks = mod
    try:
        from trn_agent_boot.trn_boot import _ntff_profile_via_ctypes
        mod.set_axon_ntff_profile_hook(
            _ntff_profile_via_ctypes("/opt/axon/libaxon_pjrt.so")
        )
    except Exception:
        pass


def _plan(nzb):
    """Compute the per-supertile weight storage layout and MM schedule.

    nzb: bool [GI, GJ] — which 32x32 blocks are present.

    Returns (plan, strip_cols):
      plan[J] = {
        'chunks': {r: (src_col_base, n_blocks)},          # DMA per row strip
        'sched':  [(r, c, woff_cols_or_None, t, start, stop)],
      }
      strip_cols[r] = total columns of strip r's DRAM panel.
    woff None => dummy matmul with the zero-weight tile (region had no blocks
    but must be initialized so the bank reduce reads defined values).
    """
    plan = []
    strip_cols = [0, 0, 0, 0]
    for J in range(N_J):
        per_strip = {r: [] for r in range(4)}   # storage order, J-major
        for j in range(J * JCOLS, (J + 1) * JCOLS):
            for i in range(GI):
                if nzb[i, j]:
                    per_strip[i % 4].append((i, j))
        chunks = {}
        queues = {}  # (r, c) -> list of (r, c, woff, t)
        for r in range(4):
            blocks = per_strip[r]
            chunks[r] = (strip_cols[r], len(blocks))
            strip_cols[r] += len(blocks) * BS
            for k, (i, j) in enumerate(blocks):
                c = j % 4
                queues.setdefault((r, c), []).append((r, c, k * BS, i // 4))
        # Regions with no blocks: one dummy MM so PSUM is zero-initialized.
        for r in range(4):
            for c in range(4):
                if (r, c) not in queues:
                    queues[(r, c)] = [(r, c, None, 0)]
        # Round-robin across the 16 sub-array positions for concurrency.
        sched = []
        qlists = [queues[k] for k in sorted(queues.keys())]
        idx = [0] * len(qlists)
        remaining = sum(len(q) for q in qlists)
        while remaining:
            for qi, q in enumerate(qlists):
                if idx[qi] < len(q):
                    r, c, woff, t = q[idx[qi]]
                    start = idx[qi] == 0
                    stop = idx[qi] == len(q) - 1
                    sched.append((r, c, woff, t, start, stop))
                    idx[qi] += 1
                    remaining -= 1
        plan.append({"chunks": chunks, "sched": sched})
    return plan, strip_cols


def _build_program(plan, strip_cols):
    import concourse.bacc as bacc
    import concourse.tile as tile
    import concourse.mybir as mybir

    nc = bacc.Bacc(debug=False)
    bf16, f32 = mybir.dt.bfloat16, mybir.dt.float32

    xt_d = nc.declare_dram_parameter("xt", [IN_F, M_CORE], bf16, isOutput=False)
    w_d = {}
    for r in range(4):
        if strip_cols[r] > 0:
            w_d[r] = nc.declare_dram_parameter(
                f"w{r}", [BS, strip_cols[r]], bf16, isOutput=False
            )
    out_d = nc.declare_dram_parameter("out", [OUT_F, M_CORE], f32, isOutput=True)

    # Largest per-(J, strip) weight chunk, in columns (>= BS for the tile alloc).
    lmax = BS
    for p in plan:
        for r in range(4):
            lmax = max(lmax, p["chunks"][r][1] * BS)

    with tile.TileContext(nc) as tc:
        with (
            tc.tile_pool(name="xp", bufs=1) as xp,
            tc.tile_pool(name="zp", bufs=1) as zp,
            tc.tile_pool(name="wp", bufs=4) as wp,
            tc.tile_pool(name="ep", bufs=4) as ep,
            tc.tile_pool(name="pp", bufs=2, space="PSUM") as pp,
        ):
            X = xp.tile([128, N_T * M_CORE], bf16)
            # Load x m-slice-major so compute on slice 0 starts ASAP.
            for m in range(N_MSL):
                for t in range(N_T):
                    nc.sync.dma_start(
                        X[:, t * M_CORE + m * MSL : t * M_CORE + (m + 1) * MSL],
                        xt_d[128 * t : 128 * (t + 1), m * MSL : (m + 1) * MSL],
                    )
            zw = zp.tile([128, BS], bf16)
            nc.vector.memset(zw[:], 0.0)

            for J in range(N_J):
                p = plan[J]
                wt = wp.tile([128, lmax], bf16, tag="wt")
                for r in range(4):
                    base, nblk = p["chunks"][r]
                    if nblk:
                        nc.sync.dma_start(
                            wt[32 * r : 32 * r + 32, : nblk * BS],
                            w_d[r][:, base : base + nblk * BS],
                        )
                for m in range(N_MSL):
                    P = pp.tile([128, 4, MSL], f32, tag="P")
                    for r, c, woff, t, start, stop in p["sched"]:
                        lhsT = (
                            zw[32 * r : 32 * r + 32, :]
                            if woff is None
                            else wt[32 * r : 32 * r + 32, woff : woff + BS]
                        )
                        nc.tensor.matmul(
                            P[32 * c : 32 * c + 32, r, :],
                            lhsT,
                            X[
                                32 * r : 32 * r + 32,
                                t * M_CORE + m * MSL : t * M_CORE + (m + 1) * MSL,
                            ],
                            start=start,
                            stop=stop,
                            tile_position=(32 * r, 32 * c),
                        )
                    ob = ep.tile([128, MSL], f32, tag="ob")
                    nc.vector.reduce_sum(
                        ob[:], P[:].transpose([0, 2, 1]), axis=mybir.AxisListType.X
                    )
                    nc.sync.dma_start(
                        out_d[128 * J : 128 * (J + 1), m * MSL : (m + 1) * MSL],
                        ob[:],
                    )
    nc.compile()
    return nc


_CACHE = {}


def kernel(x, W, bias, mask):
    assert x.shape == (B, S, IN_F) and W.shape == (IN_F, OUT_F)
    _ensure_ntff_hook()
    from concourse.bass_utils import run_bass_kernel_spmd

    # --- host-side input prep -------------------------------------------
    mask_nz = mask != 0
    nzb = mask_nz.reshape(GI, BS, GJ, BS).any(axis=(1, 3))

    key = nzb.tobytes()
    if key not in _CACHE:
        plan, strip_cols = _plan(nzb)
        nc = _build_program(plan, strip_cols)
        _CACHE[key] = (plan, strip_cols, nc)
    plan, strip_cols, nc = _CACHE[key]

    # Masked weights, gathered per row strip in storage order (J-major).
    Wm = np.where(mask_nz, W, np.float32(0)).astype(np.float32)
    W4 = Wm.reshape(GI, BS, GJ, BS)  # block (i, j) = W4[i, :, j, :]
    strips = {}
    for r in range(4):
        if strip_cols[r] == 0:
            continue
        ii, jj = [], []
        for J in range(N_J):
            for j in range(J * JCOLS, (J + 1) * JCOLS):
                for i in range(GI):
                    if nzb[i, j] and i % 4 == r:
                        ii.append(i)
                        jj.append(j)
        blk = W4[ii, :, jj, :]  # [nblk, 32, 32]
        strips[r] = np.ascontiguousarray(
            blk.transpose(1, 0, 2).reshape(BS, -1)
        ).astype(BF16)

    xf = np.ascontiguousarray(x).reshape(B * S, IN_F)
    in_maps = []
    for c in range(N_CORES):
        xt = np.ascontiguousarray(
            xf[c * M_CORE : (c + 1) * M_CORE].T
        ).astype(BF16)
        m = {"xt": xt}
        for r, arr in strips.items():
            m[f"w{r}"] = arr
        in_maps.append(m)

    # --- run -------------------------------------------------------------
    res = run_bass_kernel_spmd(nc, in_maps, list(range(N_CORES)), trace=True)

    # --- host-side output assembly --------------------------------------
    y = np.empty((B * S, OUT_F), dtype=np.float32)
    for c in range(N_CORES):
        y[c * M_CORE : (c + 1) * M_CORE] = res.results[c]["out"].T
    y = y.reshape(B, S, OUT_F)
    if np.any(bias):
        # bias is all-zero in this problem's setup; handled host-side for
        # generality.
        y = y + bias.astype(np.float32)
    kernel.last_exec_time_ns = res.exec_time_ns
    return y


# revision 5
# speedup vs baseline: 1.1699x; 1.1699x over previous
"""Block-sparse linear kernel for Trainium2 (8 NeuronCores, SPMD data-parallel).

Computes y = x @ (W * mask) + bias for
    x    [8, 1024, 4096] f32
    W    [4096, 4096]    f32
    mask [4096, 4096]    int32 (32x32-block structured, ~25% block density)
    bias [4096]          f32
    y    [8, 1024, 4096] f32

Strategy
--------
- Data parallel: core c computes rows [1024c, 1024(c+1)) of the flattened
  [8192, 4096] activation (i.e. batch element c).
- The trn2 PE array is physically 16 independent 32x32 sub-arrays; we run it
  in 64x32 tiling mode (8 concurrent sub-arrays).  The mask's 32x32 block
  granularity maps onto vertical block pairs: each present 64x32 "super
  cell" (block rows 2I,2I+1 x block col j, present if either 32x32 block is
  nonzero) becomes one K=64/M=32/N=512 matmul on sub-array
  (row_grp=I%2, col_grp=j%4); fully-zero super cells are skipped.
  Measured on hardware, the PE sustains one LDWEIGHTS+MATMUL pair per
  ~34 ns regardless of K/N, so throughput is set by the pair count —
  K=64 pairing halves it vs K=32.
- The two 64-row groups write disjoint PSUM banks, so concurrent sub-arrays
  never collide on a PSUM bank; VectorE reduces the 2 partial banks and the
  result is DMA'd out.
- Weights are gathered host-side into per-row-strip BSR-style panels (this
  mirrors the nn.Module, which stores BSR values at init), cast to bf16;
  x is transposed/cast host-side.  All matmul FLOPs run in bf16 with fp32
  PSUM accumulation (measured rel. error ~2e-3).
- The device program is compiled against the observed block pattern; it is
  exact for arbitrary masks (any block containing a nonzero mask element is
  gathered with its W*mask values; absent blocks contribute exact zeros).
"""

import numpy as np
import ml_dtypes

B, S, IN_F, OUT_F = 8, 1024, 4096, 4096
BS = 32                      # sparsity block size
GI, GJ = IN_F // BS, OUT_F // BS
GP = GI // 2                 # vertical super-rows (64 rows each)
N_CORES = 8
M_CORE = (B * S) // N_CORES  # rows of x per core (1024)
MSL = 512                    # m-slice width (one PSUM bank of fp32)
N_MSL = M_CORE // MSL        # 2
JCOLS = 4                    # output block-columns per supertile (4*32 = 128 partitions)
N_J = GJ // JCOLS            # 32 output supertiles
N_T = IN_F // 128            # 32 xT tiles

BF16 = ml_dtypes.bfloat16


def _ensure_ntff_hook():
    """Best-effort: make trace=True work under axon when the image's antenv
    lacks axon_hooks.  Harmless if it fails — tracing is skipped, results
    are still correct."""
    import sys, types
    try:
        import antenv  # noqa
    except ImportError:
        return
    try:
        from antenv.axon_hooks import get_axon_ntff_profile_hook
        if get_axon_ntff_profile_hook() is not None:
            return
        mod = sys.modules["antenv.axon_hooks"]
    except ImportError:
        mod = types.ModuleType("antenv.axon_hooks")
        mod._hook = None
        def set_axon_ntff_profile_hook(h, _m=mod):
            _m._hook = h
        def get_axon_ntff_profile_hook(_m=mod):
            return _m._hook
        mod.set_axon_ntff_profile_hook = set_axon_ntff_profile_hook
        mod.get_axon_ntff_profile_hook = get_axon_ntff_profile_hook
        sys.modules["antenv.axon_hooks"] = mod
        import antenv as _a
        _a.axon_hooks = mod
    try:
        from trn_agent_boot.trn_boot import _ntff_profile_via_ctypes
        mod.set_axon_ntff_profile_hook(
            _ntff_profile_via_ctypes("/opt/axon/libaxon_pjrt.so")
        )
    except Exception:
        pass


def _plan(nzb):
    """Per-supertile weight storage layout and MM schedule (64x32 pairing).

    nzb: bool [GI, GJ] — which 32x32 blocks are present.

    Returns (plan, strip_cols):
      plan[J] = {
        'chunks': {r2: (src_col_base, n_cells)},            # DMA per row strip
        'sched':  [(r2, c, woff_or_None, I, start, stop)],
      }
      strip_cols[r2] = total columns of strip r2's DRAM panel (r2 in {0,1}).
    woff None => dummy matmul with the zero-weight tile (region had no cells
    but must be initialized so the bank reduce reads defined values).
    """
    nzb2 = nzb[0::2] | nzb[1::2]       # [GP, GJ] supercell presence
    plan = []
    strip_cols = [0, 0]
    for J in range(N_J):
        per_strip = {0: [], 1: []}     # storage order
        for j in range(J * JCOLS, (J + 1) * JCOLS):
            for I in range(GP):
                if nzb2[I, j]:
                    per_strip[I % 2].append((I, j))
        chunks = {}
        queues = {}                    # (r2, c) -> list of (r2, c, woff, I)
        for r2 in range(2):
            cells = per_strip[r2]
            chunks[r2] = (strip_cols[r2], len(cells))
            strip_cols[r2] += len(cells) * BS
            for k, (I, j) in enumerate(cells):
                c = j % 4
                queues.setdefault((r2, c), []).append((r2, c, k * BS, I))
        for r2 in range(2):
            for c in range(4):
                if (r2, c) not in queues:
                    queues[(r2, c)] = [(r2, c, None, 0)]
        # Round-robin across the 8 sub-array positions for concurrency.
        sched = []
        qlists = [queues[k] for k in sorted(queues.keys())]
        idx = [0] * len(qlists)
        remaining = sum(len(q) for q in qlists)
        while remaining:
            for qi, q in enumerate(qlists):
                if idx[qi] < len(q):
                    r2, c, woff, I = q[idx[qi]]
                    start = idx[qi] == 0
                    stop = idx[qi] == len(q) - 1
                    sched.append((r2, c, woff, I, start, stop))
                    idx[qi] += 1
                    remaining -= 1
        plan.append({"chunks": chunks, "sched": sched})
    return plan, strip_cols


def _build_program(plan, strip_cols):
    import concourse.bacc as bacc
    import concourse.tile as tile
    import concourse.mybir as mybir

    nc = bacc.Bacc(debug=False)
    bf16, f32 = mybir.dt.bfloat16, mybir.dt.float32

    xt_d = nc.declare_dram_parameter("xt", [IN_F, M_CORE], bf16, isOutput=False)
    w_d = {}
    for r2 in range(2):
        if strip_cols[r2] > 0:
            w_d[r2] = nc.declare_dram_parameter(
                f"w{r2}", [2 * BS, strip_cols[r2]], bf16, isOutput=False
            )
    out_d = nc.declare_dram_parameter("out", [OUT_F, M_CORE], f32, isOutput=True)

    # Largest per-(J, strip) weight chunk, in columns (>= BS for the tile alloc).
    lmax = BS
    for p in plan:
        for r2 in range(2):
            lmax = max(lmax, p["chunks"][r2][1] * BS)

    with tile.TileContext(nc) as tc:
        with (
            tc.tile_pool(name="xp", bufs=1) as xp,
            tc.tile_pool(name="zp", bufs=1) as zp,
            tc.tile_pool(name="wp", bufs=4) as wp,
            tc.tile_pool(name="ep", bufs=4) as ep,
            tc.tile_pool(name="pp", bufs=3, space="PSUM") as pp,
        ):
            X = xp.tile([128, N_T * M_CORE], bf16)
            # m-slice-0 chunks first so compute starts ASAP; alternate the
            # two HWDGE queues (sync + scalar).
            for m in range(N_MSL):
                for t in range(N_T):
                    eng = nc.sync if t % 2 == 0 else nc.scalar
                    eng.dma_start(
                        X[:, t * M_CORE + m * MSL : t * M_CORE + (m + 1) * MSL],
                        xt_d[128 * t : 128 * (t + 1), m * MSL : (m + 1) * MSL],
                    )
            zw = zp.tile([128, BS], bf16)
            nc.vector.memset(zw[:], 0.0)

            for J in range(N_J):
                p = plan[J]
                wt = wp.tile([128, lmax], bf16, tag="wt")
                for r2 in range(2):
                    base, ncell = p["chunks"][r2]
                    if ncell:
                        nc.sync.dma_start(
                            wt[64 * r2 : 64 * r2 + 64, : ncell * BS],
                            w_d[r2][:, base : base + ncell * BS],
                        )
                for m in range(N_MSL):
                    P = pp.tile([128, 2, MSL], f32, tag="P")
                    for r2, c, woff, I, start, stop in p["sched"]:
                        lhsT = (
                            zw[64 * r2 : 64 * r2 + 64, :]
                            if woff is None
                            else wt[64 * r2 : 64 * r2 + 64, woff : woff + BS]
                        )
                        t = I // 2
                        xoff = t * M_CORE + m * MSL
                        nc.tensor.matmul(
                            P[32 * c : 32 * c + 32, r2, :],
                            lhsT,
                            X[64 * r2 : 64 * r2 + 64, xoff : xoff + MSL],
                            start=start,
                            stop=stop,
                            tile_position=(64 * r2, 32 * c),
                        )
                    ob = ep.tile([128, MSL], f32, tag="ob")
                    nc.vector.reduce_sum(
                        ob[:], P[:].transpose([0, 2, 1]), axis=mybir.AxisListType.X
                    )
                    eng = nc.scalar if J % 2 == 0 else nc.sync
                    eng.dma_start(
                        out_d[128 * J : 128 * (J + 1), m * MSL : (m + 1) * MSL],
                        ob[:],
                    )
    nc.compile()
    return nc


_CACHE = {}


def kernel(x, W, bias, mask):
    assert x.shape == (B, S, IN_F) and W.shape == (IN_F, OUT_F)
    _ensure_ntff_hook()
    from concourse.bass_utils import run_bass_kernel_spmd

    # --- host-side input prep -------------------------------------------
    mask_nz = mask != 0
    nzb = np.asarray(mask_nz.reshape(GI, BS, GJ, BS).any(axis=(1, 3)))

    key = nzb.tobytes()
    if key not in _CACHE:
        plan, strip_cols = _plan(nzb)
        nc = _build_program(plan, strip_cols)
        _CACHE[key] = (plan, strip_cols, nc)
    plan, strip_cols, nc = _CACHE[key]

    # Masked weights, gathered per row strip in storage order (J-major).
    # Wm's zeros for absent 32x32 blocks make half-present 64x32 panels
    # correct with no special-casing.
    Wm = np.where(mask_nz, W, np.float32(0)).astype(np.float32)
    W4 = Wm.reshape(GI, BS, GJ, BS)  # block (i, j) = W4[i, :, j, :]
    nzb2 = nzb[0::2] | nzb[1::2]
    strips = {}
    for r2 in range(2):
        if strip_cols[r2] == 0:
            continue
        II, JJ = [], []
        for J in range(N_J):
            for j in range(J * JCOLS, (J + 1) * JCOLS):
                for I in range(GP):
                    if nzb2[I, j] and I % 2 == r2:
                        II.append(I)
                        JJ.append(j)
        II = np.asarray(II)
        JJ = np.asarray(JJ)
        top = W4[2 * II, :, JJ, :]       # [n, 32, 32]
        bot = W4[2 * II + 1, :, JJ, :]   # [n, 32, 32]
        panel = np.concatenate([top, bot], axis=1)  # [n, 64, 32]
        strips[r2] = np.ascontiguousarray(
            panel.transpose(1, 0, 2).reshape(2 * BS, -1)
        ).astype(BF16)

    xf = np.ascontiguousarray(x).reshape(B * S, IN_F)
    in_maps = []
    for c in range(N_CORES):
        xt = np.ascontiguousarray(
            xf[c * M_CORE : (c + 1) * M_CORE].T
        ).astype(BF16)
        m = {"xt": xt}
        for r2, arr in strips.items():
            m[f"w{r2}"] = arr
        in_maps.append(m)

    # --- run -------------------------------------------------------------
    res = run_bass_kernel_spmd(nc, in_maps, list(range(N_CORES)), trace=True)

    # --- host-side output assembly --------------------------------------
    y = np.empty((B * S, OUT_F), dtype=np.float32)
    for c in range(N_CORES):
        y[c * M_CORE : (c + 1) * M_CORE] = res.results[c]["out"].T
    y = y.reshape(B, S, OUT_F)
    if np.any(bias):
        # bias is all-zero in this problem's setup; handled host-side for
        # generality.
        y = y + bias.astype(np.float32)
    kernel.last_exec_time_ns = res.exec_time_ns
    return y


# revision 10
# speedup vs baseline: 1.2764x; 1.0911x over previous
"""Block-sparse linear kernel for Trainium2 (8 NeuronCores, SPMD data-parallel).

Computes y = x @ (W * mask) + bias for
    x    [8, 1024, 4096] f32
    W    [4096, 4096]    f32
    mask [4096, 4096]    int32 (32x32-block structured, ~25% block density)
    bias [4096]          f32
    y    [8, 1024, 4096] f32

Strategy
--------
- Data parallel: core c computes rows [1024c, 1024(c+1)) of the flattened
  [8192, 4096] activation (i.e. batch element c).
- The trn2 PE array is physically 16 independent 32x32 sub-arrays; we run it
  in 64x32 tiling mode (8 concurrent sub-arrays).  The mask's 32x32 block
  granularity maps onto vertical block pairs: each present 64x32 "super
  cell" (block rows 2I,2I+1 x block col j, present if either 32x32 block is
  nonzero) becomes one K=64/M=32/N=512 matmul on sub-array
  (row_grp=I%2, col_grp=j%4); fully-zero super cells are skipped.
  Measured on hardware, the PE sustains one LDWEIGHTS+MATMUL pair per
  ~34 ns regardless of K/N, so throughput is set by the pair count —
  K=64 pairing halves it vs K=32.
- The two 64-row groups write disjoint PSUM banks, so concurrent sub-arrays
  never collide on a PSUM bank; VectorE reduces the 2 partial banks and the
  result is DMA'd out.
- Weights are gathered host-side into per-row-strip BSR-style panels (this
  mirrors the nn.Module, which stores BSR values at init), cast to bf16;
  x is transposed/cast host-side.  All matmul FLOPs run in bf16 with fp32
  PSUM accumulation (measured rel. error ~2e-3).
- The device program is compiled against the observed block pattern; it is
  exact for arbitrary masks (any block containing a nonzero mask element is
  gathered with its W*mask values; absent blocks contribute exact zeros).
"""

import numpy as np
import ml_dtypes

B, S, IN_F, OUT_F = 8, 1024, 4096, 4096
BS = 32                      # sparsity block size
GI, GJ = IN_F // BS, OUT_F // BS
GP = GI // 2                 # vertical super-rows (64 rows each)
N_CORES = 8
M_CORE = (B * S) // N_CORES  # rows of x per core (1024)
MSL = 512                    # m-slice width (one PSUM bank of fp32)
N_MSL = M_CORE // MSL        # 2
JCOLS = 4                    # output block-columns per supertile (4*32 = 128 partitions)
N_J = GJ // JCOLS            # 32 output supertiles
N_T = IN_F // 128            # 32 xT tiles

BF16 = ml_dtypes.bfloat16


def _ensure_ntff_hook():
    """Best-effort: make trace=True work under axon when the image's antenv
    lacks axon_hooks.  Harmless if it fails — tracing is skipped, results
    are still correct."""
    import sys, types
    try:
        import antenv  # noqa
    except ImportError:
        return
    try:
        from antenv.axon_hooks import get_axon_ntff_profile_hook
        if get_axon_ntff_profile_hook() is not None:
            return
        mod = sys.modules["antenv.axon_hooks"]
    except ImportError:
        mod = types.ModuleType("antenv.axon_hooks")
        mod._hook = None
        def set_axon_ntff_profile_hook(h, _m=mod):
            _m._hook = h
        def get_axon_ntff_profile_hook(_m=mod):
            return _m._hook
        mod.set_axon_ntff_profile_hook = set_axon_ntff_profile_hook
        mod.get_axon_ntff_profile_hook = get_axon_ntff_profile_hook
        sys.modules["antenv.axon_hooks"] = mod
        import antenv as _a
        _a.axon_hooks = mod
    try:
        from trn_agent_boot.trn_boot import _ntff_profile_via_ctypes
        mod.set_axon_ntff_profile_hook(
            _ntff_profile_via_ctypes("/opt/axon/libaxon_pjrt.so")
        )
    except Exception:
        pass


def _pair_permutation(nzb):
    """Order block-rows so vertically-paired rows co-occur in many columns.

    Greedy max-weight matching on C[a,b] = #columns where blocks a and b are
    both present; each matched pair becomes one 64-row super-row, so high
    weight = fewer half-empty 64x32 panels = fewer matmuls.
    """
    C = nzb.astype(np.int32) @ nzb.astype(np.int32).T
    pairs = []
    order = np.argsort(
        C[np.triu_indices(GI, k=1)]
    )[::-1]
    iu = np.triu_indices(GI, k=1)
    used = np.zeros(GI, dtype=bool)
    for idx in order:
        a, b = iu[0][idx], iu[1][idx]
        if not used[a] and not used[b]:
            used[a] = used[b] = True
            pairs.append((int(a), int(b)))
            if len(pairs) == GI // 2:
                break
    perm = []
    for a, b in pairs:
        perm.extend((a, b))
    for a in range(GI):      # safety for odd leftovers
        if a not in perm:
            perm.append(a)
    return np.asarray(perm)


def _plan(nzb):
    """Per-supertile weight storage layout and MM schedule (64x32 pairing).

    nzb: bool [GI, GJ] — which 32x32 blocks are present (in permuted row
    order).

    Returns (plan, strip_cols):
      plan[J] = {
        'chunks': {r2: (src_col_base, n_cells)},            # DMA per row strip
        'sched':  [(r2, c, woff_or_None, I, start, stop)],
      }
      strip_cols[r2] = total columns of strip r2's DRAM panel (r2 in {0,1}).
    woff None => dummy matmul with the zero-weight tile (region had no cells
    but must be initialized so the bank reduce reads defined values).
    """
    nzb2 = nzb[0::2] | nzb[1::2]       # [GP, GJ] supercell presence
    plan = []
    strip_cols = [0, 0]
    for J in range(N_J):
        per_strip = {0: [], 1: []}     # storage order
        for j in range(J * JCOLS, (J + 1) * JCOLS):
            for I in range(GP):
                if nzb2[I, j]:
                    per_strip[I % 2].append((I, j))
        chunks = {}
        queues = {}                    # (r2, c) -> list of (r2, c, woff, I)
        for r2 in range(2):
            cells = per_strip[r2]
            chunks[r2] = (strip_cols[r2], len(cells))
            strip_cols[r2] += len(cells) * BS
            for k, (I, j) in enumerate(cells):
                c = j % 4
                queues.setdefault((r2, c), []).append((r2, c, k * BS, I))
        for r2 in range(2):
            for c in range(4):
                if (r2, c) not in queues:
                    queues[(r2, c)] = [(r2, c, None, 0)]
        # Round-robin across the 8 sub-array positions for concurrency.
        sched = []
        qlists = [queues[k] for k in sorted(queues.keys())]
        idx = [0] * len(qlists)
        remaining = sum(len(q) for q in qlists)
        while remaining:
            for qi, q in enumerate(qlists):
                if idx[qi] < len(q):
                    r2, c, woff, I = q[idx[qi]]
                    start = idx[qi] == 0
                    stop = idx[qi] == len(q) - 1
                    sched.append((r2, c, woff, I, start, stop))
                    idx[qi] += 1
                    remaining -= 1
        plan.append({"chunks": chunks, "sched": sched})
    return plan, strip_cols


def _build_program(plan, strip_cols):
    import concourse.bacc as bacc
    import concourse.tile as tile
    import concourse.mybir as mybir

    nc = bacc.Bacc(debug=False)
    bf16, f32 = mybir.dt.bfloat16, mybir.dt.float32

    xt_d = nc.declare_dram_parameter("xt", [IN_F, M_CORE], bf16, isOutput=False)
    w_d = {}
    for r2 in range(2):
        if strip_cols[r2] > 0:
            w_d[r2] = nc.declare_dram_parameter(
                f"w{r2}", [2 * BS, strip_cols[r2]], bf16, isOutput=False
            )
    out_d = nc.declare_dram_parameter("out", [OUT_F, M_CORE], f32, isOutput=True)

    # Largest per-(J, strip) weight chunk, in columns (>= BS for the tile alloc).
    lmax = BS
    for p in plan:
        for r2 in range(2):
            lmax = max(lmax, p["chunks"][r2][1] * BS)

    N_PRE = 4  # supertiles whose weights load before x

    with tile.TileContext(nc) as tc:
        with (
            tc.tile_pool(name="xp", bufs=1) as xp,
            tc.tile_pool(name="zp", bufs=1) as zp,
            tc.tile_pool(name="wp", bufs=6) as wp,
            tc.tile_pool(name="ep", bufs=8) as ep,
            tc.tile_pool(name="pp", bufs=4, space="PSUM") as pp,
        ):
            def load_w(J):
                wt = wp.tile([128, lmax], bf16, tag="wt")
                for r2 in range(2):
                    base, ncell = plan[J]["chunks"][r2]
                    if ncell:
                        nc.sync.dma_start(
                            wt[64 * r2 : 64 * r2 + 64, : ncell * BS],
                            w_d[r2][:, base : base + ncell * BS],
                        )
                return wt

            # DMA emission order (per in-order queue): the first supertiles'
            # weights lead, then x (m-slice 0 first, split across both HWDGE
            # queues), then the remaining weights stream behind.
            wts = {J: load_w(J) for J in range(N_PRE)}
            X = xp.tile([128, N_T * M_CORE], bf16)
            for m in range(N_MSL):
                for t in range(N_T):
                    eng = nc.sync if t % 2 == 0 else nc.scalar
                    eng.dma_start(
                        X[:, t * M_CORE + m * MSL : t * M_CORE + (m + 1) * MSL],
                        xt_d[128 * t : 128 * (t + 1), m * MSL : (m + 1) * MSL],
                    )
            zw = zp.tile([128, BS], bf16)
            nc.vector.memset(zw[:], 0.0)
            for J in range(N_PRE, N_J):
                wts[J] = load_w(J)

            for J in range(N_J):
                p = plan[J]
                wt = wts[J]
                for m in range(N_MSL):
                    P = pp.tile([128, 2, MSL], f32, tag="P")
                    for r2, c, woff, I, start, stop in p["sched"]:
                        lhsT = (
                            zw[64 * r2 : 64 * r2 + 64, :]
                            if woff is None
                            else wt[64 * r2 : 64 * r2 + 64, woff : woff + BS]
                        )
                        t = I // 2
                        xoff = t * M_CORE + m * MSL
                        nc.tensor.matmul(
                            P[32 * c : 32 * c + 32, r2, :],
                            lhsT,
                            X[64 * r2 : 64 * r2 + 64, xoff : xoff + MSL],
                            start=start,
                            stop=stop,
                            tile_position=(64 * r2, 32 * c),
                        )
                    ob = ep.tile([128, MSL], f32, tag="ob")
                    nc.vector.reduce_sum(
                        ob[:], P[:].transpose([0, 2, 1]), axis=mybir.AxisListType.X
                    )
                    nc.scalar.dma_start(
                        out_d[128 * J : 128 * (J + 1), m * MSL : (m + 1) * MSL],
                        ob[:],
                    )
    nc.compile()
    return nc


_CACHE = {}


def kernel(x, W, bias, mask):
    assert x.shape == (B, S, IN_F) and W.shape == (IN_F, OUT_F)
    _ensure_ntff_hook()
    from concourse.bass_utils import run_bass_kernel_spmd

    # --- host-side input prep -------------------------------------------
    mask_nz = mask != 0
    nzb = np.asarray(mask_nz.reshape(GI, BS, GJ, BS).any(axis=(1, 3)))

    key = nzb.tobytes()
    if key not in _CACHE:
        perm = _pair_permutation(nzb)
        plan, strip_cols = _plan(nzb[perm])
        nc = _build_program(plan, strip_cols)
        _CACHE[key] = (perm, plan, strip_cols, nc)
    perm, plan, strip_cols, nc = _CACHE[key]
    nzb_p = nzb[perm]

    # Masked weights, gathered per row strip in storage order (J-major).
    # Wm's zeros for absent 32x32 blocks make half-present 64x32 panels
    # correct with no special-casing.
    Wm = np.where(mask_nz, W, np.float32(0)).astype(np.float32)
    W4 = Wm.reshape(GI, BS, GJ, BS)  # block (i, j) = W4[i, :, j, :]
    nzb2 = nzb_p[0::2] | nzb_p[1::2]
    strips = {}
    for r2 in range(2):
        if strip_cols[r2] == 0:
            continue
        II, JJ = [], []
        for J in range(N_J):
            for j in range(J * JCOLS, (J + 1) * JCOLS):
                for I in range(GP):
                    if nzb2[I, j] and I % 2 == r2:
                        II.append(I)
                        JJ.append(j)
        II = np.asarray(II)
        JJ = np.asarray(JJ)
        top = W4[perm[2 * II], :, JJ, :]       # [n, 32, 32]
        bot = W4[perm[2 * II + 1], :, JJ, :]   # [n, 32, 32]
        panel = np.concatenate([top, bot], axis=1)  # [n, 64, 32]
        strips[r2] = np.ascontiguousarray(
            panel.transpose(1, 0, 2).reshape(2 * BS, -1)
        ).astype(BF16)

    xf = np.ascontiguousarray(x).reshape(B * S, IN_F)
    in_maps = []
    for c in range(N_CORES):
        xt = np.ascontiguousarray(
            xf[c * M_CORE : (c + 1) * M_CORE].T
        ).astype(BF16)
        xt = xt.reshape(GI, BS, M_CORE)[perm].reshape(IN_F, M_CORE)
        m = {"xt": np.ascontiguousarray(xt)}
        for r2, arr in strips.items():
            m[f"w{r2}"] = arr
        in_maps.append(m)

    # --- run -------------------------------------------------------------
    res = run_bass_kernel_spmd(nc, in_maps, list(range(N_CORES)), trace=True)

    # --- host-side output assembly --------------------------------------
    y = np.empty((B * S, OUT_F), dtype=np.float32)
    for c in range(N_CORES):
        y[c * M_CORE : (c + 1) * M_CORE] = res.results[c]["out"].T
    y = y.reshape(B, S, OUT_F)
    if np.any(bias):
        # bias is all-zero in this problem's setup; handled host-side for
        # generality.
        y = y + bias.astype(np.float32)
    kernel.last_exec_time_ns = res.exec_time_ns
    return y
